# revision 45
# baseline (speedup 1.0000x reference)
# GQA attention layer (B=1, S=2048, HID=2560, H=32, HKV=8, D=128) on 8 TRN2
# NeuronCores. Tensor-parallel over kv-head groups: core c owns kv head c and
# its 4 query heads (Wq/Wk/Wv row shards, Wo column shard). The o_proj
# partials are combined with an on-device ReduceScatter over the sequence
# axis; the host reassembles the sequence-sharded outputs.
#
# Per-core dataflow (all matmuls bf16 -> fp32 PSUM):
#   1. QKV projection from X^T tiles (s-major output layout), per-head
#      RMSNorm + RoPE on DVE, PE-transpose of Q/K into [d, s] layout.
#   2. Scores are computed transposed (S^T[k, q] = K Q^T) so that the
#      P^T @ V matmul needs no transpose of the 16.8M-element prob matrix.
#      exp() on the scalar engine in 2-PSUM-bank batches (no max
#      subtraction: |scores| is bounded). Scores and PV matmuls are
#      software-pipelined so the PE never waits on the exp stream.
#      Softmax denominators: pairwise kt-tree accumulation of the exp
#      tiles on DVE, then a single all-ones [128,128] stationary matmul
#      per unit broadcasts the partition-sums to every output row; one
#      DVE reciprocal + multiply normalizes the PV output.
#   3. o_proj per 512-row chunk (DVE evictions only - the scalar engine
#      stays exp-only in phase 2, avoiding ACT table reloads). The first
#      3 chunks ReduceScatter at 512 rows; the last chunk scatters per
#      128-row subtile so the final RS tail is ~4x shorter.
import sys

if "/opt/trn_rl_repo" not in sys.path:
    sys.path.insert(0, "/opt/trn_rl_repo")

import numpy as np
import ml_dtypes

import concourse.bacc as bacc
import concourse.mybir as mybir
import concourse.tile as tile
from concourse import bass_utils, masks

BF16 = mybir.dt.bfloat16
F32 = mybir.dt.float32

B, S, HID = 1, 2048, 2560
H, HKV, D = 32, 8, 128
G = H // HKV  # q heads per kv head (= per core)
NC = 8  # cores
DQ = G * D  # per-core q width (512)
EPS = 1e-6
SCALE = 1.0 / float(np.sqrt(D))

ST = 128          # s positions per compute tile
N_ST = S // ST    # 16
HC = HID // 128   # 20 contraction chunks
XL = 256          # s positions per X^T DMA load tile
N_XL = S // XL    # 8
QC = 512          # q positions per attention unit
N_QC = S // QC    # 4
N_KT = S // 128   # 16 k tiles per attention unit
NP = N_KT // 2    # 8 score/exp pairs per unit
NO = HID // 512   # 5 o_proj free-dim chunks
MR = 128          # rows per mini-ReduceScatter (last chunk)

_NC_CACHE = None


def _build(reps: int = 1, single: bool = False):
    nc = bacc.Bacc(
        "TRN2", target_bir_lowering=False, debug=False,
        num_devices=(1 if single else NC),
    )

    xt_d = nc.dram_tensor("xt", [N_XL, HC, 128, XL], BF16, kind="ExternalInput").ap()
    wqkv_d = nc.dram_tensor(
        "wqkv", [HC, 128, DQ + 2 * D], BF16, kind="ExternalInput"
    ).ap()
    wo_d = nc.dram_tensor("wo", [G, 128, HID], BF16, kind="ExternalInput").ap()
    # fused per-st cos/sin tables: (cos*qw, sin*roll(qw), cos*kw, sin*roll(kw))
    cs_d = nc.dram_tensor("cs4", [N_ST, 4, 128, D], F32, kind="ExternalInput").ap()
    out_d = nc.dram_tensor("out", [S // NC, HID], BF16, kind="ExternalOutput").ap()

    with tile.TileContext(nc) as tc:
        with (
            tc.tile_pool(name="const", bufs=1) as cpool,
            tc.tile_pool(name="xt", bufs=3) as xt_pool,
            tc.tile_pool(name="cs", bufs=3) as cs_pool,
            tc.tile_pool(name="qw", bufs=4) as qw_pool,
            tc.tile_pool(name="kw", bufs=6) as kw_pool,
            tc.tile_pool(name="ro", bufs=2) as ro_pool,
            tc.tile_pool(name="sm", bufs=4) as sm_pool,
            tc.tile_pool(name="ep", bufs=2) as ep_pool,
            tc.tile_pool(name="tr", bufs=2) as tr_pool,
            tc.tile_pool(name="ot", bufs=8) as ot_pool,
            tc.tile_pool(name="ob", bufs=3) as ob_pool,
            tc.tile_pool(name="psA", bufs=4, space="PSUM") as psA,
            tc.tile_pool(name="psB", bufs=2, space="PSUM") as psB,
            tc.tile_pool(name="psY", bufs=2, space="PSUM") as psY,
            tc.tile_pool(name="dram", bufs=1, space="DRAM") as dram,
        ):
            for _rep in range(reps):
                # ---- resident constants / weights ----
                ident = cpool.tile([128, 128], BF16, tag="ident")
                masks.make_identity(nc, ident[:])
                # all-ones stationary: one sums matmul per unit yields the
                # softmax denominator replicated across all 128 partitions
                ones_k = cpool.tile([128, 128], BF16, tag="ones_k")
                nc.vector.memset(ones_k[:], 1.0)

                # keep the sync-engine DMA issue stream short: each
                # dma_start costs ~650ns of serial issue time, so weights
                # are one fused wqkv DMA per chunk, xt[0] four 5-chunk
                # pieces, and cos/sin one fused DMA per st-tile
                xt_t = xt_pool.tile([128, HC, XL], BF16, tag="xt")
                wqkv_t = []
                cs_pre = []
                for ch in range(HC):
                    w1 = cpool.tile([128, DQ + 2 * D], BF16, tag=f"wqkv{ch}")
                    nc.sync.dma_start(w1[:], wqkv_d[ch])
                    wqkv_t.append(w1)
                    if ch % 5 == 0:
                        j = ch // 5
                        nc.sync.dma_start(
                            xt_t[:, 5 * j : 5 * j + 5, :],
                            xt_d[0, 5 * j : 5 * j + 5].rearrange("c p s -> p c s"),
                        )
                    if ch in (1, 6):
                        ct = cs_pool.tile([128, 4, D], F32, tag="cs", name="ct")
                        nc.sync.dma_start(
                            ct[:], cs_d[ch // 5].rearrange("f p d -> p f d")
                        )
                        cs_pre.append(ct)
                    # xt[1] in two halves, slotted between xt[0] pieces so
                    # each lands just before the PE needs it
                    if ch == 6:
                        xt_next = xt_pool.tile([128, HC, XL], BF16, tag="xt")
                        nc.sync.dma_start(
                            xt_next[:, 0:10, :],
                            xt_d[1, 0:10].rearrange("c p s -> p c s"),
                        )
                    if ch == 12:
                        nc.sync.dma_start(
                            xt_next[:, 10:20, :],
                            xt_d[1, 10:20].rearrange("c p s -> p c s"),
                        )

                qt_sb = cpool.tile([128, G, S], BF16, tag="qt")   # Q^T  [d, h, s]
                kt_sb = cpool.tile([128, S], BF16, tag="kt")      # K^T  [d, s]
                v_sb = cpool.tile([128, N_KT, D], BF16, tag="v")  # V    [s%128, kt, d]

                # ================= phase 1: QKV + norm + rope + transpose ======
                for st in range(N_ST):
                    if st % (XL // ST) == 0 and st > 0:
                        # rotate to the prefetched tile and prefetch one ahead
                        xt_t = xt_next
                        k = st // (XL // ST) + 1
                        if k < N_XL:
                            xt_next = xt_pool.tile([128, HC, XL], BF16, tag="xt")
                            nc.sync.dma_start(
                                xt_next[:], xt_d[k].rearrange("c p s -> p c s")
                            )
                    soff = (st % (XL // ST)) * ST

                    if st < 2:
                        cs_t = cs_pre[st]
                    else:
                        cs_t = cs_pool.tile([128, 4, D], F32, tag="cs")
                        nc.sync.dma_start(
                            cs_t[:], cs_d[st].rearrange("f p d -> p f d")
                        )
                    cwq_t = cs_t[:, 0, :]
                    swq_t = cs_t[:, 1, :]
                    cwk_t = cs_t[:, 2, :]
                    swk_t = cs_t[:, 3, :]

                    q_ps = psA.tile([128, DQ], F32, tag="s")
                    kv_ps = psA.tile([128, DQ], F32, tag="s")
                    for ch in range(HC):
                        lhs = xt_t[:, ch, soff : soff + ST]
                        nc.tensor.matmul(
                            q_ps[:], lhs, wqkv_t[ch][:, 0:DQ],
                            start=(ch == 0), stop=(ch == HC - 1),
                        )
                        nc.tensor.matmul(
                            kv_ps[:, 0 : 2 * D], lhs, wqkv_t[ch][:, DQ : DQ + 2 * D],
                            start=(ch == 0), stop=(ch == HC - 1),
                        )

                    # evictions (scalar engine; phase 1 is Copy-table only)
                    q_sb = qw_pool.tile([128, DQ], F32, tag="qw")
                    nc.scalar.copy(q_sb[:], q_ps[:])
                    k_sb = kw_pool.tile([128, D], F32, tag="kw")
                    nc.scalar.copy(k_sb[:], kv_ps[:, 0:D])
                    nc.scalar.copy(v_sb[:, st, :], kv_ps[:, D : 2 * D])

                    # ---- RMSNorm (per head) ----
                    sq = qw_pool.tile([128, DQ], F32, tag="qw")
                    nc.vector.tensor_mul(sq[:], q_sb[:], q_sb[:])
                    ssq = sm_pool.tile([128, G + 1], F32, tag="sm")
                    nc.vector.tensor_reduce(
                        ssq[:, 0:G], sq[:].rearrange("p (h d) -> p h d", d=D),
                        axis=mybir.AxisListType.X, op=mybir.AluOpType.add,
                    )
                    ksq = kw_pool.tile([128, D], F32, tag="kw")
                    nc.vector.tensor_mul(ksq[:], k_sb[:], k_sb[:])
                    nc.vector.tensor_reduce(
                        ssq[:, G : G + 1], ksq[:].unsqueeze(1),
                        axis=mybir.AxisListType.X, op=mybir.AluOpType.add,
                    )
                    var = sm_pool.tile([128, G + 1], F32, tag="sm")
                    nc.vector.tensor_scalar(
                        var[:], ssq[:], 1.0 / D, EPS,
                        op0=mybir.AluOpType.mult, op1=mybir.AluOpType.add,
                    )
                    rt = sm_pool.tile([128, G + 1], F32, tag="sm")
                    nc.scalar.sqrt(rt[:], var[:])
                    rq = sm_pool.tile([128, G + 1], F32, tag="sm")
                    nc.vector.reciprocal(rq[:], rt[:])
                    rk = rq

                    # ---- normalize + rope (DVE) ----
                    qn = qw_pool.tile([128, DQ], F32, tag="qw")
                    qn3 = qn[:].rearrange("p (h d) -> p h d", d=D)
                    nc.vector.tensor_tensor(
                        qn3, q_sb[:].rearrange("p (h d) -> p h d", d=D),
                        rq[:, 0:G].unsqueeze(2).to_broadcast([128, G, D]),
                        op=mybir.AluOpType.mult,
                    )
                    t1 = qw_pool.tile([128, DQ], F32, tag="qw")
                    t13 = t1[:].rearrange("p (h d) -> p h d", d=D)
                    cwq3 = cwq_t[:].unsqueeze(1).to_broadcast([128, G, D])
                    swq3 = swq_t[:].unsqueeze(1).to_broadcast([128, G, D])
                    nc.vector.tensor_tensor(t13, qn3, cwq3, op=mybir.AluOpType.mult)
                    u = qw_pool.tile([128, DQ], F32, tag="qw")
                    u3 = u[:].rearrange("p (h d) -> p h d", d=D)
                    hd = D // 2
                    nc.vector.tensor_tensor(
                        u3[:, :, 0:hd], qn3[:, :, hd:D], swq3[:, :, 0:hd],
                        op=mybir.AluOpType.mult,
                    )
                    nc.vector.tensor_tensor(
                        u3[:, :, hd:D], qn3[:, :, 0:hd], swq3[:, :, hd:D],
                        op=mybir.AluOpType.mult,
                    )
                    qro = ro_pool.tile([128, DQ], BF16, tag="qro")
                    qro3 = qro[:].rearrange("p (h d) -> p h d", d=D)
                    nc.vector.tensor_sub(qro3[:, :, 0:hd], t13[:, :, 0:hd], u3[:, :, 0:hd])
                    nc.vector.tensor_add(qro3[:, :, hd:D], t13[:, :, hd:D], u3[:, :, hd:D])

                    kn = kw_pool.tile([128, D], F32, tag="kw")
                    nc.vector.tensor_tensor(
                        kn[:], k_sb[:],
                        rk[:, G : G + 1].to_broadcast([128, D]),
                        op=mybir.AluOpType.mult,
                    )
                    kt1 = kw_pool.tile([128, D], F32, tag="kw")
                    nc.vector.tensor_tensor(kt1[:], kn[:], cwk_t[:], op=mybir.AluOpType.mult)
                    ku = kw_pool.tile([128, D], F32, tag="kw")
                    nc.vector.tensor_tensor(
                        ku[:, 0:hd], kn[:, hd:D], swk_t[:, 0:hd], op=mybir.AluOpType.mult
                    )
                    nc.vector.tensor_tensor(
                        ku[:, hd:D], kn[:, 0:hd], swk_t[:, hd:D], op=mybir.AluOpType.mult
                    )
                    kro = ro_pool.tile([128, D], BF16, tag="kro")
                    nc.vector.tensor_sub(kro[:, 0:hd], kt1[:, 0:hd], ku[:, 0:hd])
                    nc.vector.tensor_add(kro[:, hd:D], kt1[:, hd:D], ku[:, hd:D])

                    # ---- transpose Q heads + K into [d, s] ----
                    for h in range(G):
                        tp = psY.tile([128, 128], BF16, tag="y", name="tp")
                        nc.tensor.transpose(tp[:], qro[:, h * D : (h + 1) * D], ident[:])
                        nc.scalar.copy(qt_sb[:, h, st * ST : (st + 1) * ST], tp[:])
                    tp = psY.tile([128, 128], BF16, tag="y")
                    nc.tensor.transpose(tp[:], kro[:], ident[:])
                    nc.scalar.copy(kt_sb[:, st * ST : (st + 1) * ST], tp[:])

                # ================= phase 2: attention + o_proj + RS ============
                # wo is first needed ~20us into phase 2; load it behind the
                # phase-1 traffic instead of ahead of it
                wo_sb = cpool.tile([128, G, HID], BF16, tag="wo")
                nc.sync.dma_start(wo_sb[:], wo_d.rearrange("c p n -> p c n"))

                # Slot pipeline over 16 attention units u = (qc, h):
                #   slot t: per-kt interleave of scores+exp+L1(u_t), pv(u_{t-1}),
                #   and the y-groups of one o_proj 128-row si-block (chunk-lagged
                #   by 1 slot past the chunk boundary so the last ot is ready).
                #   ones/recip/ot(u_{t-1}) at slot end; tree L2-4 of u_t on
                #   gpsimd (latency-tolerant: result needed a full slot later).
                #   The exp stream (scalar) and the sums tree never gate the PE;
                #   ob bufs=3 + psY bufs=2 ride out DMA outages while RS wire
                #   traffic hogs the queues.
                NU = N_QC * G  # 16 units
                RROWS = QC // NC  # 64 output rows per core per RS chunk
                unit_state = [None] * NU  # [ep, pv_ps, tr1, colsum]
                ot_tiles = {}
                rs_tiles = {}

                def emit_scores_kt(t, kt):
                    ep, _, tr1 = unit_state[t][:3]
                    qsl = qt_sb[:, t % G, (t // G) * QC : (t // G + 1) * QC]
                    s_ps = psA.tile([128, QC], F32, tag="s")
                    nc.tensor.matmul(
                        s_ps[:],
                        kt_sb[:, kt * 128 : (kt + 1) * 128],
                        qsl, start=True, stop=True,
                    )
                    nc.scalar.activation(
                        ep[:, kt, :], s_ps[:],
                        mybir.ActivationFunctionType.Exp, scale=SCALE,
                    )
                    if kt % 2 == 1:
                        # kt-tree level 1 on DVE
                        tr = tr_pool.tile([128, QC], BF16, tag="t1", bufs=9)
                        nc.vector.tensor_add(
                            tr[:], ep[:, kt - 1, :], ep[:, kt, :]
                        )
                        tr1.append(tr)

                ob_blk = {}

                # RS piece key -> (first si-block, #si-blocks, out_d row)
                RS_PIECES = {
                    0: (0, 4, 0), 1: (4, 4, 64),
                    2: (8, 2, 128), 5: (10, 2, 160),  # chunk 2 in halves
                    3: (12, 4, 192),
                }
                B2KEY = {}
                for _k, (_b0, _nb, _) in RS_PIECES.items():
                    for _b in range(_b0, _b0 + _nb):
                        B2KEY[_b] = _k

                def emit_y_group(b, no):
                    qc_o, si = b // 4, b % 4
                    key = B2KEY[b]
                    b0, nb, _ = RS_PIECES[key]
                    rs_si = b - b0
                    if rs_si == 0 and no == 0:
                        rs_in = dram.tile([nb * ST, HID], BF16, tag=f"rsin{key}", name="rs_in")
                        rs_out = dram.tile([nb * ST // NC, HID], BF16, tag=f"rsout{key}", name="rs_out")
                        rs_tiles[key] = (rs_in, rs_out)
                    if no == 0:
                        ob_blk[b] = ob_pool.tile([128, HID], BF16, tag="ob", name="ob")
                    rs_in, rs_out = rs_tiles[key]
                    ob = ob_blk[b]
                    y_ps = psY.tile([128, 512], F32, tag="y")
                    for h in range(G):
                        nc.tensor.matmul(
                            y_ps[:],
                            ot_tiles[qc_o * G + h][:, si * ST : (si + 1) * ST],
                            wo_sb[:, h, no * 512 : (no + 1) * 512],
                            start=(h == 0), stop=(h == G - 1),
                        )
                    nc.vector.tensor_copy(ob[:, no * 512 : (no + 1) * 512], y_ps[:])
                    nc.sync.dma_start(
                        rs_in[rs_si * ST : (rs_si + 1) * ST, no * 512 : (no + 1) * 512],
                        ob[:, no * 512 : (no + 1) * 512],
                    )

                def emit_rs(key):
                    rs_in, rs_out = rs_tiles[key]
                    _, nb, orow = RS_PIECES[key]
                    r = nb * ST // NC
                    if single:
                        nc.sync.dma_start(out_d[orow : orow + r, :], rs_in[0:r, :])
                        return
                    nc.gpsimd.collective_compute(
                        "ReduceScatter",
                        mybir.AluOpType.add,
                        replica_groups=[list(range(NC))],
                        ins=[rs_in.opt()],
                        outs=[rs_out.opt()],
                    )
                    nc.sync.dma_start(out_d[orow : orow + r, :], rs_out[:])

                for t in range(NU + 2):
                    if t < NU:
                        ep = ep_pool.tile([128, N_KT, QC], BF16, tag="ep")
                        unit_state[t] = [ep, None, []]
                    if 1 <= t <= NU:
                        unit_state[t - 1][1] = psB.tile(
                            [128, QC], F32, tag="b", name="pv_ps"
                        )
                    # si>=1 blocks interleave into the kt loop; si=0 blocks
                    # append at the end of slot 4qc+4 (their last ot input is
                    # computed mid-slot), so RS(qc) fires at slot 4qc+7.
                    # Staging stays spread out: bursts of staging DMA
                    # alongside RS wire traffic starve the collective.
                    STD = (2, 5, 8, 11, 14)
                    islots = {
                        5: [(1, STD)], 6: [(2, STD)], 7: [(3, STD)],
                        9: [(5, STD)], 10: [(6, STD)], 11: [(7, STD)],
                        13: [(9, STD)], 14: [(10, STD)], 15: [(11, STD)],
                    }
                    kt_blocks = {}
                    for b, kts in islots.get(t, []):
                        for i, kt in enumerate(kts):
                            kt_blocks.setdefault(kt, []).append((b, i))
                    # ---- per-kt interleave ----
                    if t <= NU:
                        for kt in range(N_KT):
                            if t < NU:
                                emit_scores_kt(t, kt)
                            if t >= 1:
                                epm, pvm = unit_state[t - 1][:2]
                                nc.tensor.matmul(
                                    pvm[:], v_sb[:, kt, :], epm[:, kt, :],
                                    start=(kt == 0), stop=(kt == N_KT - 1),
                                )
                            for b, no in kt_blocks.get(kt, []):
                                emit_y_group(b, no)
                    # ---- finish unit u_{t-1}: ones-mm, recip, ot ----
                    if 1 <= t <= NU:
                        epm, pvm, _, colsum = unit_state[t - 1]
                        sums_ps = psY.tile([128, QC], F32, tag="y", name="sums_ps")
                        nc.tensor.matmul(
                            sums_ps[:], ones_k[:], colsum[:], start=True, stop=True
                        )
                        rb = sm_pool.tile([128, QC], F32, tag="rb", bufs=2)
                        nc.vector.reciprocal_approx_fast(rb[:], sums_ps[:])
                        ot = ot_pool.tile([128, QC], BF16, tag="ot")
                        nc.vector.tensor_tensor(
                            ot[:], pvm[:], rb[:], op=mybir.AluOpType.mult
                        )
                        ot_tiles[t - 1] = ot
                    # ---- si=0 block of chunk (t-4)//4 appended at slot end ----
                    if t in (4, 8, 12, 16):
                        for no in range(NO):
                            emit_y_group(t - 4, no)
                    if t == 16:
                        for no in range(NO):
                            emit_y_group(13, no)
                    # ---- epilogue: last chunk's si=2..3 blocks ----
                    if t == NU + 1:
                        for b in range(NU - 2, NU):
                            for no in range(NO):
                                emit_y_group(b, no)
                    # ---- kt-tree levels 2..4 of u_t on gpsimd ----
                    if t < NU:
                        tr1 = unit_state[t][2]
                        tr2 = []
                        for j in range(4):
                            tr = tr_pool.tile([128, QC], BF16, tag="t2", bufs=5)
                            nc.gpsimd.tensor_add(tr[:], tr1[2 * j][:], tr1[2 * j + 1][:])
                            tr2.append(tr)
                        tr3 = []
                        for j in range(2):
                            tr = tr_pool.tile([128, QC], BF16, tag="t3", bufs=4)
                            nc.gpsimd.tensor_add(tr[:], tr2[2 * j][:], tr2[2 * j + 1][:])
                            tr3.append(tr)
                        colsum = tr_pool.tile([128, QC], BF16, tag="t4", bufs=4)
                        nc.gpsimd.tensor_add(colsum[:], tr3[0][:], tr3[1][:])
                        unit_state[t].append(colsum)
                    # ---- RS triggers: each piece as soon as staged ----
                    if t == 7:
                        emit_rs(0)
                    elif t == 11:
                        emit_rs(1)
                    elif t == 13:
                        emit_rs(2)
                    elif t == 15:
                        emit_rs(5)
                    elif t == NU + 1:
                        emit_rs(3)

    nc.compile()
    return nc


def _get_nc():
    global _NC_CACHE
    if _NC_CACHE is None:
        _NC_CACHE = _build()
    return _NC_CACHE


def make_in_maps(inputs):
    X = np.asarray(inputs["hidden_states"], dtype=np.float32).reshape(S, HID)
    freqs = np.asarray(inputs["freqs_cis"], dtype=np.float32)
    Wq = np.asarray(inputs["Wq"], dtype=np.float32)
    Wk = np.asarray(inputs["Wk"], dtype=np.float32)
    Wv = np.asarray(inputs["Wv"], dtype=np.float32)
    Wo = np.asarray(inputs["Wo"], dtype=np.float32)
    qw = np.asarray(inputs["q_norm_w"], dtype=np.float32)
    kw = np.asarray(inputs["k_norm_w"], dtype=np.float32)

    bf = ml_dtypes.bfloat16
    # X^T load tiles: (L, ch, p, s) = X[L*XL+s, ch*128+p]
    xt = np.ascontiguousarray(
        X.reshape(N_XL, XL, HC, 128).transpose(0, 2, 3, 1).astype(bf)
    )
    cos, sin = freqs[0], freqs[1]  # [S, D]
    cwq = (cos * qw[None, :]).reshape(N_ST, 128, D)
    swq = (sin * np.roll(qw, D // 2)[None, :]).reshape(N_ST, 128, D)
    cwk = (cos * kw[None, :]).reshape(N_ST, 128, D)
    swk = (sin * np.roll(kw, D // 2)[None, :]).reshape(N_ST, 128, D)
    cs4 = np.ascontiguousarray(
        np.stack([cwq, swq, cwk, swk], axis=1).astype(np.float32)
    )  # [N_ST, 4, 128, D]

    in_maps = []
    for c in range(NC):
        wq_c = Wq[c * DQ : (c + 1) * DQ, :]  # [DQ, HID]
        wk_c = Wk[c * D : (c + 1) * D, :]
        wv_c = Wv[c * D : (c + 1) * D, :]
        wqkv_t = np.ascontiguousarray(
            np.concatenate([wq_c.T, wk_c.T, wv_c.T], axis=1)
            .reshape(HC, 128, DQ + 2 * D)
            .astype(bf)
        )
        wo_c = Wo[:, c * DQ : (c + 1) * DQ]  # [HID, DQ]
        wo_t = np.ascontiguousarray(wo_c.T.reshape(G, 128, HID).astype(bf))
        in_maps.append(
            {
                "xt": xt,
                "wqkv": wqkv_t,
                "wo": wo_t,
                "cs4": cs4,
            }
        )
    return in_maps


def assemble(outs):
    # outs[c] is [S//NC, HID] bf16. RS piece covering global rows
    # [g0, g0+R*NC) gives core c rows [g0 + R*c, +R), stored at
    # core-local rows [l0, +R). Chunk 2 is scattered in two halves.
    pieces = [  # (global row, local row, rows per core)
        (0, 0, 64), (512, 64, 64),
        (1024, 128, 32), (1280, 160, 32),
        (1536, 192, 64),
    ]
    y = np.empty((S, HID), dtype=np.float32)
    for g0, l0, r in pieces:
        for c in range(NC):
            y[g0 + r * c : g0 + r * (c + 1), :] = outs[c][l0 : l0 + r, :].astype(
                np.float32
            )
    return y.reshape(B, S, HID)


def kernel(**inputs) -> np.ndarray:
    nc = _get_nc()
    in_maps = make_in_maps(inputs)
    res = bass_utils.run_bass_kernel_spmd(nc, in_maps, core_ids=list(range(NC)))
    return assemble([r["out"] for r in res.results])
